# revision 9
# baseline (speedup 1.0000x reference)
"""Block-sparse attention (local + vertical-strided causal mask) on 8 TRN2 cores.

Sharding: one head per NeuronCore (H=8, n_cores=8).

Per-core device algorithm (head h, residue r = 7-h):
  The 4096x4096 score matrix is processed at 128x128 granularity:
  "pair" i = q block-rows (2i, 2i+1) (128 q tokens), "chunk" = 128 k tokens
  (2 mask blocks of 64). Local window -> chunks c in [i-8, i] of K itself;
  vertical-strided blocks -> host-gathered K_vert (6 blocks of 64, kb = 8j+r),
  processed as 3 chunks shared by all cores, with per-core validity applied
  as multiplicative 0/1 per-partition scalars.

  S^T orientation: S^T[k,q] = kT_chunk.T @ qT_pair  (PE, bf16)
  P^T = exp(sm_scale * S^T)                          (ACT, one call per PSUM group)
  masks (triangle / window-start / vert validity)    (DVE)
  out[q,0:128] += P^T_chunk.T @ [V | 1]_chunk        (PE, PSUM-accumulated)
  col 128 of out = softmax denominator; normalize with per-partition
  reciprocal + tensor_scalar multiply, DMA out [q, d] per pair.
"""

import numpy as np
import ml_dtypes

BF16 = ml_dtypes.bfloat16

H = 8
S = 4096
D = 128
BLK = 64
NB = S // BLK        # 64 block rows
NPAIR = NB // 2      # 32 row pairs
NVSLOT = 6           # usable vertical slots (kb = 8j + r <= 47)
NVC = NVSLOT // 2    # 3 vertical chunks
GROUP = 8            # PSUM staging slots per exp group (8 * 128 f32 = 2 banks)

NEG = -30000.0


def make_schedule():
    """Global ordered visit list. visit = (kind, idx, pair)
    kind "local": idx = chunk c (k blocks 2c, 2c+1), pairs i in [c, c+8]
    kind "vert":  idx = vc (K_vert slots 2vc, 2vc+1)
    Vert visits for pairs [c0, c0+8) are inserted right before local chunk
    c0 in {8, 16, 24}, after all their opening local chunks."""
    visits = []
    for c in range(NPAIR):
        if c in (8, 16, 24):
            for vc in range(NVC):
                if 8 * vc + 8 <= c:
                    for i in range(c, c + 8):
                        visits.append(("vert", vc, i))
        for i in range(c, min(c + 8, NPAIR - 1) + 1):
            visits.append(("local", c, i))
    return visits


def vert_visit_order():
    return [(vc_, i_) for (kind, vc_, i_) in make_schedule() if kind == "vert"]


_PROGRAM = None


def _build_program(loop_n=None):
    """Build the SPMD program. loop_n: wrap the whole body (incl. input DMA)
    in an in-NEFF For loop with that trip count — used only for timing."""
    import contextlib
    import concourse.bass as bass
    import concourse.mybir as mybir
    import concourse.tile as tile
    from concourse import bacc

    fp32 = mybir.dt.float32
    bf16 = mybir.dt.bfloat16

    nc = bacc.Bacc("TRN2", target_bir_lowering=False, debug=False, num_devices=H)

    qt_d = nc.dram_tensor("qt", [D, S], bf16, kind="ExternalInput").ap()
    kt_d = nc.dram_tensor("kt", [D, S], bf16, kind="ExternalInput").ap()
    ktv_d = nc.dram_tensor("ktv", [D, NVSLOT * BLK], bf16, kind="ExternalInput").ap()
    vaug_d = nc.dram_tensor("vaug", [128, NPAIR, D + 1], bf16, kind="ExternalInput").ap()
    vvaug_d = nc.dram_tensor("vvaug", [128, NVC, D + 1], bf16, kind="ExternalInput").ap()
    vs_d = nc.dram_tensor("vs", [128, 48, 2], fp32, kind="ExternalInput").ap()
    tri_d = nc.dram_tensor("tri", [128, 128], bf16, kind="ExternalInput").ap()
    mstart_d = nc.dram_tensor("mstart", [128, 128], bf16, kind="ExternalInput").ap()
    smsc_d = nc.dram_tensor("smsc", [128, 1], fp32, kind="ExternalInput").ap()
    o_d = nc.dram_tensor("o", [S, D], fp32, kind="ExternalOutput").ap()

    visits = make_schedule()
    # first/last visit index per pair
    first = {}
    last = {}
    for g, (kind, idx, i) in enumerate(visits):
        first.setdefault(i, g)
        last[i] = g
    # PSUM start_tensor_calc zeroes the full 2KB bank (zero-region), so only
    # the first matmul touching an oacc tile may carry start=True.
    tile_first = {}
    for g, (kind, idx, i) in enumerate(visits):
        tile_first.setdefault(i // 3, g)
    n_groups = (len(visits) + GROUP - 1) // GROUP

    with tile.TileContext(nc) as tc:
        with (
            tc.tile_pool(name="big", bufs=1) as big,
            tc.tile_pool(name="stage", bufs=2, space="PSUM") as stagep,
            tc.tile_pool(name="oacc", bufs=4, space="PSUM") as oaccp,
            tc.tile_pool(name="pt", bufs=3) as ptp,
            tc.tile_pool(name="ob", bufs=3) as obp,
            tc.tile_pool(name="rd", bufs=4) as rdp,
        ):
            if loop_n is not None:
                loop_cm = tc.For_i(
                    0,
                    loop_n,
                    hint_engines=(
                        mybir.EngineType.PE,
                        mybir.EngineType.DVE,
                        mybir.EngineType.Activation,
                        mybir.EngineType.Pool,
                        mybir.EngineType.SP,
                    ),
                )
            else:
                loop_cm = contextlib.nullcontext()
            with loop_cm:
                _emit_body(nc, tc, locals())
    nc.compile()
    return nc


def _emit_body(nc, tc, env):
    import concourse.mybir as mybir

    fp32 = mybir.dt.float32
    bf16 = mybir.dt.bfloat16
    big, stagep, oaccp, ptp, obp, rdp = (
        env["big"], env["stagep"], env["oaccp"], env["ptp"], env["obp"], env["rdp"]
    )
    qt_d, kt_d, ktv_d, vaug_d, vvaug_d, vs_d, tri_d, mstart_d, smsc_d, o_d = (
        env["qt_d"], env["kt_d"], env["ktv_d"], env["vaug_d"], env["vvaug_d"],
        env["vs_d"], env["tri_d"], env["mstart_d"], env["smsc_d"], env["o_d"],
    )
    visits, first, last, tile_first, n_groups = (
        env["visits"], env["first"], env["last"], env["tile_first"], env["n_groups"]
    )
    if True:
        if True:
            qt = big.tile([D, S], bf16)
            kt = big.tile([D, S], bf16)
            ktv = big.tile([D, NVSLOT * BLK], bf16)
            vaug = big.tile([128, NPAIR, D + 1], bf16)
            vvaug = big.tile([128, NVC, D + 1], bf16)
            vs = big.tile([128, 48, 2], fp32)
            tri = big.tile([128, 128], bf16)
            mstart = big.tile([128, 128], bf16)
            smsc = big.tile([128, 1], fp32)

            # split big loads for DMA queue parallelism
            for a in range(8):
                sl = slice(a * (S // 8), (a + 1) * (S // 8))
                nc.sync.dma_start(out=qt[:, sl], in_=qt_d[:, sl])
                nc.sync.dma_start(out=kt[:, sl], in_=kt_d[:, sl])
                sl4 = slice(a * (NPAIR // 8), (a + 1) * (NPAIR // 8))
                nc.sync.dma_start(out=vaug[:, sl4], in_=vaug_d[:, sl4])
            nc.sync.dma_start(out=ktv[:], in_=ktv_d[:])
            nc.sync.dma_start(out=vvaug[:], in_=vvaug_d[:])
            nc.sync.dma_start(out=vs[:], in_=vs_d[:])
            nc.sync.dma_start(out=tri[:], in_=tri_d[:])
            nc.sync.dma_start(out=mstart[:], in_=mstart_d[:])
            nc.sync.dma_start(out=smsc[:], in_=smsc_d[:])

            oacc_tiles = {}  # pair-group (i//3) -> psum tile [128, 3, 129]
            v_idx = 0  # running vertical-visit index (matches host vs layout)

            for gi in range(n_groups):
                gvis = visits[gi * GROUP : (gi + 1) * GROUP]
                n = len(gvis)
                stage = stagep.tile([128, GROUP * 128], fp32, tag="stage")
                ptt = ptp.tile([128, GROUP * 128], bf16, tag="pt")

                # --- S^T matmuls, batched over runs of consecutive pairs
                # sharing one k-chunk, split at 4-slot (one PSUM bank) bounds.
                # start=True only on the first run per bank (bank zero-region).
                s = 0
                seen_banks = set()
                while s < n:
                    kind, idx, i0 = gvis[s]
                    e = s + 1
                    while (
                        e < n
                        and e % 4 != 0
                        and gvis[e][0] == kind
                        and gvis[e][1] == idx
                        and gvis[e][2] == gvis[e - 1][2] + 1
                    ):
                        e += 1
                    ln = e - s
                    lhsT = (
                        kt[:, idx * 128 : (idx + 1) * 128]
                        if kind == "local"
                        else ktv[:, idx * 128 : (idx + 1) * 128]
                    )
                    bank = s // 4
                    nc.tensor.matmul(
                        stage[:, s * 128 : e * 128],
                        lhsT,
                        qt[:, i0 * 128 : (i0 + ln) * 128],
                        start=bank not in seen_banks,
                        stop=True,
                        skip_group_check=True,
                    )
                    seen_banks.add(bank)
                    s = e

                # --- one exp for the whole group
                nc.scalar.activation(
                    out=ptt[:, : n * 128],
                    in_=stage[:, : n * 128],
                    func=mybir.ActivationFunctionType.Exp,
                    scale=smsc[:, 0:1],
                )

                # --- masks
                for s, (kind, idx, i) in enumerate(gvis):
                    sl = slice(s * 128, (s + 1) * 128)
                    if kind == "local" and idx == i:
                        nc.vector.tensor_mul(ptt[:, sl], ptt[:, sl], tri[:])
                    elif kind == "local" and idx == i - 8:
                        nc.vector.tensor_mul(ptt[:, sl], ptt[:, sl], mstart[:])
                    elif kind == "vert":
                        for hh in range(2):
                            hsl = slice(s * 128 + hh * 64, s * 128 + (hh + 1) * 64)
                            nc.vector.tensor_scalar_mul(
                                ptt[:, hsl], ptt[:, hsl], vs[:, v_idx, hh : hh + 1]
                            )
                        v_idx += 1

                # --- PV matmuls + epilogue
                for s, (kind, idx, i) in enumerate(gvis):
                    g = gi * GROUP + s
                    pg = i // 3
                    if pg not in oacc_tiles:
                        oacc_tiles[pg] = oaccp.tile([128, 3, D + 1], fp32, tag="oacc", name=f"oacc{pg}")
                    oacc = oacc_tiles[pg]
                    rhs = vaug[:, idx] if kind == "local" else vvaug[:, idx]
                    nc.tensor.matmul(
                        oacc[:, i % 3],
                        ptt[:, s * 128 : (s + 1) * 128],
                        rhs,
                        start=(g == tile_first[i // 3]),
                        stop=(g == last[i]),
                        skip_group_check=True,
                    )
                    if g == last[i]:
                        rd = rdp.tile([128, 1], fp32, tag="rd")
                        nc.vector.reciprocal(rd[:], oacc[:, i % 3, D : D + 1])
                        ob = obp.tile([128, D], fp32, tag="ob")
                        nc.vector.tensor_scalar_mul(ob[:], oacc[:, i % 3, 0:D], rd[:])
                        nc.sync.dma_start(
                            out=o_d[i * 128 : (i + 1) * 128, :], in_=ob[:]
                        )


def _get_program():
    global _PROGRAM
    if _PROGRAM is None:
        _PROGRAM = _build_program()
    return _PROGRAM


def _host_inputs(q, k, v, sm_scale):
    """Per-core input dicts (host-side shard + layout)."""
    q = np.asarray(q, dtype=np.float32)
    k = np.asarray(k, dtype=np.float32)
    v = np.asarray(v, dtype=np.float32)
    smv = float(np.asarray(sm_scale, dtype=np.float32))

    tri = np.zeros((128, 128), dtype=BF16)
    p = np.arange(128)
    tri[p[:, None] <= p[None, :]] = BF16(1.0)
    mstart = np.zeros((128, 128), dtype=BF16)
    mstart[64:, :64] = BF16(1.0)
    smsc = np.full((128, 1), smv, dtype=np.float32)

    vorder = vert_visit_order()
    ins = []
    for h in range(H):
        r = 7 - h
        qh, kh, vh = q[0, h], k[0, h], v[0, h]
        qt = np.ascontiguousarray(qh.T).astype(BF16)
        kt = np.ascontiguousarray(kh.T).astype(BF16)
        vblocks = [8 * j + r for j in range(NVSLOT)]
        kv = np.concatenate([kh[b * BLK : (b + 1) * BLK] for b in vblocks], axis=0)
        ktv = np.ascontiguousarray(kv.T).astype(BF16)
        vaug = np.concatenate(
            [vh, np.ones((S, 1), np.float32)], axis=1
        ).astype(BF16)  # [4096, 129]
        vaug = np.ascontiguousarray(
            vaug.reshape(NPAIR, 128, D + 1).transpose(1, 0, 2)
        )  # [128, 32, 129]
        vv = np.concatenate([vh[b * BLK : (b + 1) * BLK] for b in vblocks], axis=0)
        vvaug = np.concatenate([vv, np.ones((NVSLOT * BLK, 1), np.float32)], axis=1)
        vvaug = np.ascontiguousarray(
            vvaug.astype(BF16).reshape(NVC, 128, D + 1).transpose(1, 0, 2)
        )  # [128, 3, 129]

        vsc = np.zeros((128, 48, 2), dtype=np.float32)
        for vi, (vc, i) in enumerate(vorder):
            for hh in range(2):
                qb = 2 * i + hh
                slot = 2 * vc + (p >= 64).astype(np.int64)  # per-partition slot
                kb = 8 * slot + r
                vsc[:, vi, hh] = (kb <= qb - 16).astype(np.float32)
        ins.append(
            dict(
                qt=qt, kt=kt, ktv=ktv, vaug=vaug, vvaug=vvaug,
                vs=vsc, tri=tri, mstart=mstart, smsc=smsc,
            )
        )
    return ins


def kernel(q, k, v, sm_scale):
    from concourse.bass_utils import run_bass_kernel_spmd

    nc = _get_program()
    ins = _host_inputs(q, k, v, sm_scale)
    res = run_bass_kernel_spmd(nc, ins, core_ids=list(range(H)))
    out = np.stack([res.results[h]["o"] for h in range(H)], axis=0)[None]
    return out.astype(np.float32)


# revision 10
# speedup vs baseline: 1.2985x; 1.2985x over previous
"""Block-sparse attention (local + vertical-strided causal mask) on 8 TRN2 cores.

Sharding: one head per NeuronCore (H=8, n_cores=8).

Per-core device algorithm (head h, residue r = 7-h):
  The 4096x4096 score matrix is processed at 128x128 granularity:
  "pair" i = q block-rows (2i, 2i+1) (128 q tokens), "chunk" = 128 k tokens
  (2 mask blocks of 64). Local window -> chunks c in [i-8, i] of K itself;
  vertical-strided blocks -> host-gathered K_vert (6 blocks of 64, kb = 8j+r),
  processed as 3 chunks shared by all cores, with per-core validity applied
  as multiplicative 0/1 per-partition scalars.

  S^T orientation: S^T[k,q] = kT_chunk.T @ qT_pair  (PE, bf16)
  P^T = exp(sm_scale * S^T)                          (ACT, one call per PSUM group)
  masks (triangle / window-start / vert validity)    (DVE)
  out[q,0:128] += P^T_chunk.T @ [V | 1]_chunk        (PE, PSUM-accumulated)
  col 128 of out = softmax denominator; normalize with per-partition
  reciprocal + tensor_scalar multiply, DMA out [q, d] per pair.
"""

import numpy as np
import ml_dtypes

BF16 = ml_dtypes.bfloat16

H = 8
S = 4096
D = 128
BLK = 64
NB = S // BLK        # 64 block rows
NPAIR = NB // 2      # 32 row pairs
NVSLOT = 6           # usable vertical slots (kb = 8j + r <= 47)
NVC = NVSLOT // 2    # 3 vertical chunks
GROUP = 8            # PSUM staging slots per exp group (8 * 128 f32 = 2 banks)

NEG = -30000.0


def make_schedule():
    """Global ordered visit list. visit = (kind, idx, pair)
    kind "local": idx = chunk c (k blocks 2c, 2c+1), pairs i in [c, c+8]
    kind "vert":  idx = vc (K_vert slots 2vc, 2vc+1)
    Vert visits for pairs [c0, c0+8) are inserted right before local chunk
    c0 in {8, 16, 24}, after all their opening local chunks."""
    visits = []
    for c in range(NPAIR):
        if c in (8, 16, 24):
            for vc in range(NVC):
                if 8 * vc + 8 <= c:
                    for i in range(c, c + 8):
                        visits.append(("vert", vc, i))
        for i in range(c, min(c + 8, NPAIR - 1) + 1):
            visits.append(("local", c, i))
    return visits


def vert_visit_order():
    return [(vc_, i_) for (kind, vc_, i_) in make_schedule() if kind == "vert"]


_PROGRAM = None


def _build_program(loop_n=None):
    """Build the SPMD program. loop_n: wrap the whole body (incl. input DMA)
    in an in-NEFF For loop with that trip count — used only for timing."""
    import contextlib
    import concourse.bass as bass
    import concourse.mybir as mybir
    import concourse.tile as tile
    from concourse import bacc

    fp32 = mybir.dt.float32
    bf16 = mybir.dt.bfloat16

    nc = bacc.Bacc("TRN2", target_bir_lowering=False, debug=False, num_devices=H)

    qt_d = nc.dram_tensor("qt", [D, S], bf16, kind="ExternalInput").ap()
    kt_d = nc.dram_tensor("kt", [D, S], bf16, kind="ExternalInput").ap()
    ktv_d = nc.dram_tensor("ktv", [D, NVSLOT * BLK], bf16, kind="ExternalInput").ap()
    vaug_d = nc.dram_tensor("vaug", [128, NPAIR, D + 1], bf16, kind="ExternalInput").ap()
    vvaug_d = nc.dram_tensor("vvaug", [128, NVC, D + 1], bf16, kind="ExternalInput").ap()
    vs_d = nc.dram_tensor("vs", [128, 48, 2], fp32, kind="ExternalInput").ap()
    tri_d = nc.dram_tensor("tri", [128, 128], bf16, kind="ExternalInput").ap()
    mstart_d = nc.dram_tensor("mstart", [128, 128], bf16, kind="ExternalInput").ap()
    smsc_d = nc.dram_tensor("smsc", [128, 1], fp32, kind="ExternalInput").ap()
    o_d = nc.dram_tensor("o", [S, D], fp32, kind="ExternalOutput").ap()

    visits = make_schedule()
    # first/last visit index per pair
    first = {}
    last = {}
    for g, (kind, idx, i) in enumerate(visits):
        first.setdefault(i, g)
        last[i] = g
    # PSUM start_tensor_calc zeroes the full 2KB bank (zero-region), so only
    # the first matmul touching an oacc tile may carry start=True.
    tile_first = {}
    for g, (kind, idx, i) in enumerate(visits):
        tile_first.setdefault(i // 3, g)
    n_groups = (len(visits) + GROUP - 1) // GROUP

    with tile.TileContext(nc) as tc:
        with (
            tc.tile_pool(name="big", bufs=1) as big,
            tc.tile_pool(name="stage", bufs=2, space="PSUM") as stagep,
            tc.tile_pool(name="oacc", bufs=4, space="PSUM") as oaccp,
            tc.tile_pool(name="pt", bufs=3) as ptp,
            tc.tile_pool(name="ob", bufs=3) as obp,
            tc.tile_pool(name="rd", bufs=4) as rdp,
        ):
            if loop_n is not None:
                loop_cm = tc.For_i(
                    0,
                    loop_n,
                    hint_engines=(
                        mybir.EngineType.PE,
                        mybir.EngineType.DVE,
                        mybir.EngineType.Activation,
                        mybir.EngineType.Pool,
                        mybir.EngineType.SP,
                    ),
                )
            else:
                loop_cm = contextlib.nullcontext()
            with loop_cm:
                _emit_body(nc, tc, locals())
    nc.compile()
    return nc


def _emit_body(nc, tc, env):
    import concourse.mybir as mybir

    fp32 = mybir.dt.float32
    bf16 = mybir.dt.bfloat16
    big, stagep, oaccp, ptp, obp, rdp = (
        env["big"], env["stagep"], env["oaccp"], env["ptp"], env["obp"], env["rdp"]
    )
    qt_d, kt_d, ktv_d, vaug_d, vvaug_d, vs_d, tri_d, mstart_d, smsc_d, o_d = (
        env["qt_d"], env["kt_d"], env["ktv_d"], env["vaug_d"], env["vvaug_d"],
        env["vs_d"], env["tri_d"], env["mstart_d"], env["smsc_d"], env["o_d"],
    )
    visits, first, last, tile_first, n_groups = (
        env["visits"], env["first"], env["last"], env["tile_first"], env["n_groups"]
    )
    if True:
        if True:
            qt = big.tile([D, S], bf16)
            kt = big.tile([D, S], bf16)
            ktv = big.tile([D, NVSLOT * BLK], bf16)
            vaug = big.tile([128, NPAIR, D + 1], bf16)
            vvaug = big.tile([128, NVC, D + 1], bf16)
            vs = big.tile([128, 48, 2], fp32)
            tri = big.tile([128, 128], bf16)
            mstart = big.tile([128, 128], bf16)
            smsc = big.tile([128, 1], fp32)

            # split big loads for DMA queue parallelism
            for a in range(8):
                sl = slice(a * (S // 8), (a + 1) * (S // 8))
                nc.sync.dma_start(out=qt[:, sl], in_=qt_d[:, sl])
                nc.sync.dma_start(out=kt[:, sl], in_=kt_d[:, sl])
                sl4 = slice(a * (NPAIR // 8), (a + 1) * (NPAIR // 8))
                nc.sync.dma_start(out=vaug[:, sl4], in_=vaug_d[:, sl4])
            nc.sync.dma_start(out=ktv[:], in_=ktv_d[:])
            nc.sync.dma_start(out=vvaug[:], in_=vvaug_d[:])
            nc.sync.dma_start(out=vs[:], in_=vs_d[:])
            nc.sync.dma_start(out=tri[:], in_=tri_d[:])
            nc.sync.dma_start(out=mstart[:], in_=mstart_d[:])
            nc.sync.dma_start(out=smsc[:], in_=smsc_d[:])

            oacc_tiles = {}  # pair-group (i//3) -> psum tile [128, 3, 129]
            v_idx = 0  # running vertical-visit index (matches host vs layout)
            pending_pv = None  # software pipeline: PV of group gi-1 emitted
            # after S^T of group gi so PE streams while ACT/DVE process gi-1

            for gi in range(n_groups):
                gvis = visits[gi * GROUP : (gi + 1) * GROUP]
                n = len(gvis)
                stage = stagep.tile([128, GROUP * 128], fp32, tag="stage")
                ptt = ptp.tile([128, GROUP * 128], bf16, tag="pt")

                # --- S^T matmuls, batched over runs of consecutive pairs
                # sharing one k-chunk, split at 4-slot (one PSUM bank) bounds.
                # start=True only on the first run per bank (bank zero-region).
                s = 0
                seen_banks = set()
                while s < n:
                    kind, idx, i0 = gvis[s]
                    e = s + 1
                    while (
                        e < n
                        and e % 4 != 0
                        and gvis[e][0] == kind
                        and gvis[e][1] == idx
                        and gvis[e][2] == gvis[e - 1][2] + 1
                    ):
                        e += 1
                    ln = e - s
                    lhsT = (
                        kt[:, idx * 128 : (idx + 1) * 128]
                        if kind == "local"
                        else ktv[:, idx * 128 : (idx + 1) * 128]
                    )
                    bank = s // 4
                    nc.tensor.matmul(
                        stage[:, s * 128 : e * 128],
                        lhsT,
                        qt[:, i0 * 128 : (i0 + ln) * 128],
                        start=bank not in seen_banks,
                        stop=True,
                        skip_group_check=True,
                    )
                    seen_banks.add(bank)
                    s = e

                if pending_pv is not None:
                    pending_pv()

                # --- one exp for the whole group
                nc.scalar.activation(
                    out=ptt[:, : n * 128],
                    in_=stage[:, : n * 128],
                    func=mybir.ActivationFunctionType.Exp,
                    scale=smsc[:, 0:1],
                )

                # --- masks
                for s, (kind, idx, i) in enumerate(gvis):
                    sl = slice(s * 128, (s + 1) * 128)
                    if kind == "local" and idx == i:
                        nc.vector.tensor_mul(ptt[:, sl], ptt[:, sl], tri[:])
                    elif kind == "local" and idx == i - 8:
                        nc.vector.tensor_mul(ptt[:, sl], ptt[:, sl], mstart[:])
                    elif kind == "vert":
                        for hh in range(2):
                            hsl = slice(s * 128 + hh * 64, s * 128 + (hh + 1) * 64)
                            nc.vector.tensor_scalar_mul(
                                ptt[:, hsl], ptt[:, hsl], vs[:, v_idx, hh : hh + 1]
                            )
                        v_idx += 1

                # --- PV matmuls + epilogue (deferred one group)
                def make_pv(gi, gvis, ptt):
                    def emit_pv():
                        for s, (kind, idx, i) in enumerate(gvis):
                            g = gi * GROUP + s
                            pg = i // 3
                            if pg not in oacc_tiles:
                                oacc_tiles[pg] = oaccp.tile(
                                    [128, 3, D + 1], fp32, tag="oacc", name=f"oacc{pg}"
                                )
                            oacc = oacc_tiles[pg]
                            rhs = vaug[:, idx] if kind == "local" else vvaug[:, idx]
                            nc.tensor.matmul(
                                oacc[:, i % 3],
                                ptt[:, s * 128 : (s + 1) * 128],
                                rhs,
                                start=(g == tile_first[i // 3]),
                                stop=(g == last[i]),
                                skip_group_check=True,
                            )
                            if g == last[i]:
                                rd = rdp.tile([128, 1], fp32, tag="rd")
                                nc.vector.reciprocal(rd[:], oacc[:, i % 3, D : D + 1])
                                ob = obp.tile([128, D], fp32, tag="ob")
                                nc.vector.tensor_scalar_mul(
                                    ob[:], oacc[:, i % 3, 0:D], rd[:]
                                )
                                nc.sync.dma_start(
                                    out=o_d[i * 128 : (i + 1) * 128, :], in_=ob[:]
                                )
                    return emit_pv

                pending_pv = make_pv(gi, gvis, ptt)
            if pending_pv is not None:
                pending_pv()


def _get_program():
    global _PROGRAM
    if _PROGRAM is None:
        _PROGRAM = _build_program()
    return _PROGRAM


def _host_inputs(q, k, v, sm_scale):
    """Per-core input dicts (host-side shard + layout)."""
    q = np.asarray(q, dtype=np.float32)
    k = np.asarray(k, dtype=np.float32)
    v = np.asarray(v, dtype=np.float32)
    smv = float(np.asarray(sm_scale, dtype=np.float32))

    tri = np.zeros((128, 128), dtype=BF16)
    p = np.arange(128)
    tri[p[:, None] <= p[None, :]] = BF16(1.0)
    mstart = np.zeros((128, 128), dtype=BF16)
    mstart[64:, :64] = BF16(1.0)
    smsc = np.full((128, 1), smv, dtype=np.float32)

    vorder = vert_visit_order()
    ins = []
    for h in range(H):
        r = 7 - h
        qh, kh, vh = q[0, h], k[0, h], v[0, h]
        qt = np.ascontiguousarray(qh.T).astype(BF16)
        kt = np.ascontiguousarray(kh.T).astype(BF16)
        vblocks = [8 * j + r for j in range(NVSLOT)]
        kv = np.concatenate([kh[b * BLK : (b + 1) * BLK] for b in vblocks], axis=0)
        ktv = np.ascontiguousarray(kv.T).astype(BF16)
        vaug = np.concatenate(
            [vh, np.ones((S, 1), np.float32)], axis=1
        ).astype(BF16)  # [4096, 129]
        vaug = np.ascontiguousarray(
            vaug.reshape(NPAIR, 128, D + 1).transpose(1, 0, 2)
        )  # [128, 32, 129]
        vv = np.concatenate([vh[b * BLK : (b + 1) * BLK] for b in vblocks], axis=0)
        vvaug = np.concatenate([vv, np.ones((NVSLOT * BLK, 1), np.float32)], axis=1)
        vvaug = np.ascontiguousarray(
            vvaug.astype(BF16).reshape(NVC, 128, D + 1).transpose(1, 0, 2)
        )  # [128, 3, 129]

        vsc = np.zeros((128, 48, 2), dtype=np.float32)
        for vi, (vc, i) in enumerate(vorder):
            for hh in range(2):
                qb = 2 * i + hh
                slot = 2 * vc + (p >= 64).astype(np.int64)  # per-partition slot
                kb = 8 * slot + r
                vsc[:, vi, hh] = (kb <= qb - 16).astype(np.float32)
        ins.append(
            dict(
                qt=qt, kt=kt, ktv=ktv, vaug=vaug, vvaug=vvaug,
                vs=vsc, tri=tri, mstart=mstart, smsc=smsc,
            )
        )
    return ins


def kernel(q, k, v, sm_scale):
    from concourse.bass_utils import run_bass_kernel_spmd

    nc = _get_program()
    ins = _host_inputs(q, k, v, sm_scale)
    res = run_bass_kernel_spmd(nc, ins, core_ids=list(range(H)))
    out = np.stack([res.results[h]["o"] for h in range(H)], axis=0)[None]
    return out.astype(np.float32)


# revision 12
# speedup vs baseline: 4.0288x; 3.1027x over previous
"""Block-sparse attention (local + vertical-strided causal mask) on 8 TRN2 cores.

Sharding: one head per NeuronCore (H=8, n_cores=8).

Per-core device algorithm (head h, residue r = 7-h):
  The 4096x4096 score matrix is processed at 128x128 granularity:
  "pair" i = q block-rows (2i, 2i+1) (128 q tokens), "chunk" = 128 k tokens
  (2 mask blocks of 64). Local window -> chunks c in [i-8, i] of K itself;
  vertical-strided blocks -> host-gathered K_vert (6 blocks of 64, kb = 8j+r),
  processed as 3 chunks shared by all cores, with per-core validity applied
  as multiplicative 0/1 per-partition scalars.

  S^T orientation: S^T[k,q] = kT_chunk.T @ qT_pair  (PE, bf16)
  P^T = exp(sm_scale * S^T)                          (ACT, one call per PSUM group)
  masks (triangle / window-start / vert validity)    (DVE)
  out[q,0:128] += P^T_chunk.T @ [V | 1]_chunk        (PE, PSUM-accumulated)
  col 128 of out = softmax denominator; normalize with per-partition
  reciprocal + tensor_scalar multiply, DMA out [q, d] per pair.
"""

import numpy as np
import ml_dtypes

BF16 = ml_dtypes.bfloat16

H = 8
S = 4096
D = 128
BLK = 64
NB = S // BLK        # 64 block rows
NPAIR = NB // 2      # 32 row pairs
NVSLOT = 6           # usable vertical slots (kb = 8j + r <= 47)
NVC = NVSLOT // 2    # 3 vertical chunks
GROUP = 8            # PSUM staging slots per exp group (8 * 128 f32 = 2 banks)

NEG = -30000.0


def make_schedule():
    """Global ordered visit list. visit = (kind, idx, pair)
    kind "local": idx = chunk c (k blocks 2c, 2c+1), pairs i in [c, c+8]
    kind "vert":  idx = vc (K_vert slots 2vc, 2vc+1)
    Vert visits for pairs [c0, c0+8) are inserted right before local chunk
    c0 in {8, 16, 24}, after all their opening local chunks."""
    visits = []
    for c in range(NPAIR):
        if c in (8, 16, 24):
            for vc in range(NVC):
                if 8 * vc + 8 <= c:
                    for i in range(c, c + 8):
                        visits.append(("vert", vc, i))
        for i in range(c, min(c + 8, NPAIR - 1) + 1):
            visits.append(("local", c, i))
    return visits


def vert_visit_order():
    return [(vc_, i_) for (kind, vc_, i_) in make_schedule() if kind == "vert"]


_PROGRAM = None


def _build_program(loop_n=None, ablate=()):
    """Build the SPMD program. loop_n: wrap the whole body (incl. input DMA)
    in an in-NEFF For loop with that trip count — used only for timing.
    ablate: subset of {"masks","pv","epi","exp"} — drop stages (timing only)."""
    import contextlib
    import concourse.bass as bass
    import concourse.mybir as mybir
    import concourse.tile as tile
    from concourse import bacc

    fp32 = mybir.dt.float32
    bf16 = mybir.dt.bfloat16

    nc = bacc.Bacc("TRN2", target_bir_lowering=False, debug=False, num_devices=H)

    qt_d = nc.dram_tensor("qt", [D, S], bf16, kind="ExternalInput").ap()
    kt_d = nc.dram_tensor("kt", [D, S], bf16, kind="ExternalInput").ap()
    ktv_d = nc.dram_tensor("ktv", [D, NVSLOT * BLK], bf16, kind="ExternalInput").ap()
    vaug_d = nc.dram_tensor("vaug", [128, NPAIR, D + 1], bf16, kind="ExternalInput").ap()
    vvaug_d = nc.dram_tensor("vvaug", [128, NVC, D + 1], bf16, kind="ExternalInput").ap()
    vs_d = nc.dram_tensor("vs", [128, 48, 2], fp32, kind="ExternalInput").ap()
    tri_d = nc.dram_tensor("tri", [128, 128], bf16, kind="ExternalInput").ap()
    mstart_d = nc.dram_tensor("mstart", [128, 128], bf16, kind="ExternalInput").ap()
    smsc_d = nc.dram_tensor("smsc", [128, 1], fp32, kind="ExternalInput").ap()
    o_d = nc.dram_tensor("o", [S, D], fp32, kind="ExternalOutput").ap()

    visits = make_schedule()
    # first/last visit index per pair
    first = {}
    last = {}
    for g, (kind, idx, i) in enumerate(visits):
        first.setdefault(i, g)
        last[i] = g
    # PSUM start_tensor_calc zeroes the full 2KB bank (zero-region), so only
    # the first matmul touching an oacc tile may carry start=True.
    tile_first = {}
    for g, (kind, idx, i) in enumerate(visits):
        tile_first.setdefault(i // 3, g)
    n_groups = (len(visits) + GROUP - 1) // GROUP

    with tile.TileContext(nc) as tc:
        with (
            tc.tile_pool(name="big", bufs=1) as big,
            tc.tile_pool(name="stage", bufs=2, space="PSUM") as stagep,
            tc.tile_pool(name="oacc", bufs=4, space="PSUM") as oaccp,
            tc.tile_pool(name="pt", bufs=4) as ptp,
            tc.tile_pool(name="ob", bufs=3) as obp,
            tc.tile_pool(name="rd", bufs=4) as rdp,
        ):
            if loop_n is not None:
                loop_cm = tc.For_i(
                    0,
                    loop_n,
                    hint_engines=(
                        mybir.EngineType.PE,
                        mybir.EngineType.DVE,
                        mybir.EngineType.Activation,
                        mybir.EngineType.Pool,
                        mybir.EngineType.SP,
                    ),
                )
            else:
                loop_cm = contextlib.nullcontext()
            with loop_cm:
                _emit_body(nc, tc, locals(), frozenset(ablate))
    nc.compile()
    return nc


def _emit_body(nc, tc, env, ablate=frozenset()):
    import concourse.mybir as mybir

    fp32 = mybir.dt.float32
    bf16 = mybir.dt.bfloat16
    big, stagep, oaccp, ptp, obp, rdp = (
        env["big"], env["stagep"], env["oaccp"], env["ptp"], env["obp"], env["rdp"]
    )
    qt_d, kt_d, ktv_d, vaug_d, vvaug_d, vs_d, tri_d, mstart_d, smsc_d, o_d = (
        env["qt_d"], env["kt_d"], env["ktv_d"], env["vaug_d"], env["vvaug_d"],
        env["vs_d"], env["tri_d"], env["mstart_d"], env["smsc_d"], env["o_d"],
    )
    visits, first, last, tile_first, n_groups = (
        env["visits"], env["first"], env["last"], env["tile_first"], env["n_groups"]
    )
    if True:
        if True:
            qt = big.tile([D, S], bf16)
            kt = big.tile([D, S], bf16)
            ktv = big.tile([D, NVSLOT * BLK], bf16)
            vaug = big.tile([128, NPAIR, D + 1], bf16)
            vvaug = big.tile([128, NVC, D + 1], bf16)
            vs = big.tile([128, 48, 2], fp32)
            tri = big.tile([128, 128], bf16)
            mstart = big.tile([128, 128], bf16)
            smsc = big.tile([128, 1], fp32)

            # small tensors first: group 0's exp/masks/PV depend on them
            nc.sync.dma_start(out=smsc[:], in_=smsc_d[:])
            nc.sync.dma_start(out=tri[:], in_=tri_d[:])
            nc.sync.dma_start(out=mstart[:], in_=mstart_d[:])
            nc.sync.dma_start(out=vs[:], in_=vs_d[:])
            nc.sync.dma_start(out=ktv[:], in_=ktv_d[:])
            nc.sync.dma_start(out=vvaug[:], in_=vvaug_d[:])
            # big loads split fine, in first-use order (kt chunk c at local c,
            # qt pair i from chunk max(0,i-8), vaug chunk c at PV time)
            for a in range(16):
                sl = slice(a * (S // 16), (a + 1) * (S // 16))
                nc.sync.dma_start(out=kt[:, sl], in_=kt_d[:, sl])
                nc.sync.dma_start(out=qt[:, sl], in_=qt_d[:, sl])
                sl4 = slice(a * (NPAIR // 16), (a + 1) * (NPAIR // 16))
                nc.sync.dma_start(out=vaug[:, sl4], in_=vaug_d[:, sl4])

            oacc_tiles = {}  # pair-group (i//3) -> psum tile [128, 3, 129]
            v_idx = 0  # running vertical-visit index (matches host vs layout)
            pending_pv = None  # software pipeline: PV of group gi-1 emitted
            # after S^T of group gi so PE streams while ACT/DVE process gi-1

            for gi in range(n_groups):
                gvis = visits[gi * GROUP : (gi + 1) * GROUP]
                n = len(gvis)
                stage = stagep.tile([128, GROUP * 128], fp32, tag="stage")
                ptt = ptp.tile([128, GROUP * 128], bf16, tag="pt")

                # --- S^T matmuls, batched over runs of consecutive pairs
                # sharing one k-chunk, split at 4-slot (one PSUM bank) bounds.
                # start=True only on the first run per bank (bank zero-region).
                s = 0
                seen_banks = set()
                while s < n:
                    kind, idx, i0 = gvis[s]
                    e = s + 1
                    while (
                        e < n
                        and e % 4 != 0
                        and gvis[e][0] == kind
                        and gvis[e][1] == idx
                        and gvis[e][2] == gvis[e - 1][2] + 1
                    ):
                        e += 1
                    ln = e - s
                    lhsT = (
                        kt[:, idx * 128 : (idx + 1) * 128]
                        if kind == "local"
                        else ktv[:, idx * 128 : (idx + 1) * 128]
                    )
                    bank = s // 4
                    nc.tensor.matmul(
                        stage[:, s * 128 : e * 128],
                        lhsT,
                        qt[:, i0 * 128 : (i0 + ln) * 128],
                        start=bank not in seen_banks,
                        stop=True,
                        skip_group_check=True,
                    )
                    seen_banks.add(bank)
                    s = e

                if pending_pv is not None:
                    pending_pv()

                # --- one exp for the whole group
                if "exp" not in ablate:
                    nc.scalar.activation(
                        out=ptt[:, : n * 128],
                        in_=stage[:, : n * 128],
                        func=mybir.ActivationFunctionType.Exp,
                        scale=smsc[:, 0:1],
                    )

                # --- masks
                for s, (kind, idx, i) in enumerate(gvis):
                    if "masks" in ablate:
                        if kind == "vert":
                            v_idx += 1
                        continue
                    sl = slice(s * 128, (s + 1) * 128)
                    if kind == "local" and idx == i:
                        nc.vector.tensor_mul(ptt[:, sl], ptt[:, sl], tri[:])
                    elif kind == "local" and idx == i - 8:
                        nc.vector.tensor_mul(ptt[:, sl], ptt[:, sl], mstart[:])
                    elif kind == "vert":
                        for hh in range(2):
                            hsl = slice(s * 128 + hh * 64, s * 128 + (hh + 1) * 64)
                            nc.vector.tensor_scalar_mul(
                                ptt[:, hsl], ptt[:, hsl], vs[:, v_idx, hh : hh + 1]
                            )
                        v_idx += 1

                # --- PV matmuls + epilogue (deferred one group)
                def make_pv(gi, gvis, ptt):
                    def emit_pv():
                        if "pv" in ablate:
                            return
                        for s, (kind, idx, i) in enumerate(gvis):
                            g = gi * GROUP + s
                            pg = i // 3
                            if pg not in oacc_tiles:
                                oacc_tiles[pg] = oaccp.tile(
                                    [128, 3, D + 1], fp32, tag="oacc", name=f"oacc{pg}"
                                )
                            oacc = oacc_tiles[pg]
                            rhs = vaug[:, idx] if kind == "local" else vvaug[:, idx]
                            nc.tensor.matmul(
                                oacc[:, i % 3],
                                ptt[:, s * 128 : (s + 1) * 128],
                                rhs,
                                start=(g == tile_first[i // 3]),
                                stop=(g == last[i]),
                                skip_group_check=True,
                            )
                            if g == last[i] and "epi" not in ablate:
                                rd = rdp.tile([128, 1], fp32, tag="rd")
                                nc.vector.reciprocal(rd[:], oacc[:, i % 3, D : D + 1])
                                ob = obp.tile([128, D], fp32, tag="ob")
                                nc.vector.tensor_scalar_mul(
                                    ob[:], oacc[:, i % 3, 0:D], rd[:]
                                )
                                nc.gpsimd.dma_start(
                                    out=o_d[i * 128 : (i + 1) * 128, :], in_=ob[:]
                                )
                    return emit_pv

                pending_pv = make_pv(gi, gvis, ptt)
            if pending_pv is not None:
                pending_pv()


def _get_program():
    global _PROGRAM
    if _PROGRAM is None:
        _PROGRAM = _build_program()
    return _PROGRAM


def _host_inputs(q, k, v, sm_scale):
    """Per-core input dicts (host-side shard + layout)."""
    q = np.asarray(q, dtype=np.float32)
    k = np.asarray(k, dtype=np.float32)
    v = np.asarray(v, dtype=np.float32)
    smv = float(np.asarray(sm_scale, dtype=np.float32))

    tri = np.zeros((128, 128), dtype=BF16)
    p = np.arange(128)
    tri[p[:, None] <= p[None, :]] = BF16(1.0)
    mstart = np.zeros((128, 128), dtype=BF16)
    mstart[64:, :64] = BF16(1.0)
    smsc = np.full((128, 1), smv, dtype=np.float32)

    vorder = vert_visit_order()
    ins = []
    for h in range(H):
        r = 7 - h
        qh, kh, vh = q[0, h], k[0, h], v[0, h]
        qt = np.ascontiguousarray(qh.T).astype(BF16)
        kt = np.ascontiguousarray(kh.T).astype(BF16)
        vblocks = [8 * j + r for j in range(NVSLOT)]
        kv = np.concatenate([kh[b * BLK : (b + 1) * BLK] for b in vblocks], axis=0)
        ktv = np.ascontiguousarray(kv.T).astype(BF16)
        vaug = np.concatenate(
            [vh, np.ones((S, 1), np.float32)], axis=1
        ).astype(BF16)  # [4096, 129]
        vaug = np.ascontiguousarray(
            vaug.reshape(NPAIR, 128, D + 1).transpose(1, 0, 2)
        )  # [128, 32, 129]
        vv = np.concatenate([vh[b * BLK : (b + 1) * BLK] for b in vblocks], axis=0)
        vvaug = np.concatenate([vv, np.ones((NVSLOT * BLK, 1), np.float32)], axis=1)
        vvaug = np.ascontiguousarray(
            vvaug.astype(BF16).reshape(NVC, 128, D + 1).transpose(1, 0, 2)
        )  # [128, 3, 129]

        vsc = np.zeros((128, 48, 2), dtype=np.float32)
        for vi, (vc, i) in enumerate(vorder):
            for hh in range(2):
                qb = 2 * i + hh
                slot = 2 * vc + (p >= 64).astype(np.int64)  # per-partition slot
                kb = 8 * slot + r
                vsc[:, vi, hh] = (kb <= qb - 16).astype(np.float32)
        ins.append(
            dict(
                qt=qt, kt=kt, ktv=ktv, vaug=vaug, vvaug=vvaug,
                vs=vsc, tri=tri, mstart=mstart, smsc=smsc,
            )
        )
    return ins


def kernel(q, k, v, sm_scale):
    from concourse.bass_utils import run_bass_kernel_spmd

    nc = _get_program()
    ins = _host_inputs(q, k, v, sm_scale)
    res = run_bass_kernel_spmd(nc, ins, core_ids=list(range(H)))
    out = np.stack([res.results[h]["o"] for h in range(H)], axis=0)[None]
    return out.astype(np.float32)
